# revision 1
# baseline (speedup 1.0000x reference)
"""Trainium2 Bass kernel for nn_Attention_5514738008849.

Dense transformer attention block with axial rotary embeddings:
  x:(8,1024,1024) -> qkv -> rope(q,k) -> softmax(qk^T/sqrt(d)) v -> proj+bias

Sharding: pure data-parallel over batch B=8 across the 8 NeuronCores (one
batch element per core, full weights replicated). No collectives.

Per-core dataflow (all matmuls fp32r: fp32 bits, 11-bit mantissa,
1 cycle/row at N>=256 vs 4 for plain fp32):
  - x^T supplied by the host (numpy transpose in kernel()), streamed on the
    SWDGE queue in parallel with weights on HWDGE
  - Q^T,K^T = W^T x^T  (out-dim on partitions); V = x W_v (token-major)
  - rotary: pair-shuffle via a 128x128 signed-permutation matmul, then
    q_rot = q*cos + shuf*sin elementwise on DVE (pass-dims use cos=1,sin=0);
    PSUM evacuations ride on the otherwise-idle Scalar engine
  - logits^T[k,q] per head; exp on ACT (scale=1/8 folded in), software-
    pipelined across head boundaries
  - AV with a ones-column appended to V => row 64 of the psum accumulator
    holds the softmax denominator per q; normalize via DVE reciprocal +
    gpsimd partition_broadcast + tensor_tensor multiply
  - proj token-major with bias added via a K=1 ones-row matmul
"""

import os
import sys

sys.path.insert(0, "/opt/trn_rl_repo")

# This kernel needs the axon-tunneled NeuronCores. A JAX_PLATFORMS=cpu pin
# (used by some harnesses for the jax reference) would prevent the axon
# backend from registering; clearing it here is a no-op when jax has already
# initialized and restores device visibility when it hasn't.
if os.environ.get("JAX_PLATFORMS", "") not in ("", None):
    if "axon" not in os.environ["JAX_PLATFORMS"]:
        os.environ.pop("JAX_PLATFORMS", None)

import numpy as np

import concourse.bass as bass
import concourse.bacc as bacc_mod
import concourse.mybir as mybir
from concourse.bass_utils import run_bass_kernel_spmd
from concourse.tile import TileContext

B, N, C = 8, 1024, 1024
H, D = 16, 64          # heads, head dim
ROT = 32               # rotary dims per head (head_dim // 2)
FH = FW = 32           # token grid for axial rope
NCORES = 8
F32 = mybir.dt.float32
F32R = mybir.dt.float32r


def _host_tables():
    """Rotary cos/sin in d-major (dim-on-partition) layout + shuffle matrix."""
    dim_r = D // 4                                    # 16
    base = np.linspace(1.0, (FH * FW) / 2.0, dim_r // 2) * np.pi   # (8,)

    def axis_freqs(n):
        pos = np.linspace(-1.0, 1.0, n)
        f = pos[:, None] * base[None, :]              # (n, 8)
        return np.repeat(f, 2, axis=-1)               # (n, 16)

    fH = np.broadcast_to(axis_freqs(FH)[:, None, :], (FH, FW, dim_r))
    fW = np.broadcast_to(axis_freqs(FW)[None, :, :], (FH, FW, dim_r))
    freqs = np.concatenate([fH, fW], axis=-1).reshape(N, ROT)      # (1024, 32)

    # d-major table for one 128-partition block = two heads:
    # rows 0-31 rot (head even), 32-63 pass, 64-95 rot (head odd), 96-127 pass
    cos_d = np.ones((128, N), np.float32)
    sin_d = np.zeros((128, N), np.float32)
    ct = np.cos(freqs).T.astype(np.float32)           # (32, 1024)
    st = np.sin(freqs).T.astype(np.float32)
    cos_d[0:32] = ct
    cos_d[64:96] = ct
    sin_d[0:32] = st
    sin_d[64:96] = st

    # signed permutation: shuf[2i] = -q[2i+1], shuf[2i+1] = q[2i] on rot rows
    pshuf = np.zeros((128, 128), np.float32)
    for off in (0, 64):
        for i in range(ROT // 2):
            r0, r1 = off + 2 * i, off + 2 * i + 1
            pshuf[r1, r0] = -1.0                      # out[r0] = -in[r1]
            pshuf[r0, r1] = 1.0                       # out[r1] = +in[r0]

    return cos_d, sin_d, pshuf


def _build_program():
    nc = bacc_mod.Bacc()
    xt_h = nc.declare_dram_parameter("xt", [C, N], F32, isOutput=False)
    wqkv_h = nc.declare_dram_parameter("w_qkv", [C, 3 * C], F32, isOutput=False)
    wproj_h = nc.declare_dram_parameter("w_proj", [C, C], F32, isOutput=False)
    brow_h = nc.declare_dram_parameter("b_row", [1, C], F32, isOutput=False)
    cos_h = nc.declare_dram_parameter("cos_d", [128, N], F32, isOutput=False)
    sin_h = nc.declare_dram_parameter("sin_d", [128, N], F32, isOutput=False)
    pshuf_h = nc.declare_dram_parameter("pshuf", [128, 128], F32, isOutput=False)
    ones_h = nc.declare_dram_parameter("ones_row", [1, 128], F32, isOutput=False)
    onescol_h = nc.declare_dram_parameter("ones_col", [128, 16], F32, isOutput=False)
    out_h = nc.declare_dram_parameter("out", [N, C], F32, isOutput=True)

    def f32r(ap):
        return ap.bitcast(F32R)

    with nc.allow_low_precision(reason="fp32r (11-bit mantissa) operands"), \
         TileContext(nc) as tc, \
         tc.tile_pool(name="consts", bufs=1) as consts, \
         tc.tile_pool(name="big", bufs=1) as big:
        cos_sb = consts.tile([128, N], F32)
        sin_sb = consts.tile([128, N], F32)
        pshuf_sb = consts.tile([128, 128], F32)
        brow_sb = consts.tile([1, C], F32)
        ones_sb = consts.tile([1, 128], F32)
        nc.sync.dma_start(out=cos_sb, in_=cos_h[:, :])
        nc.sync.dma_start(out=sin_sb, in_=sin_h[:, :])
        nc.sync.dma_start(out=f32r(pshuf_sb), in_=f32r(pshuf_h[:, :]))
        nc.sync.dma_start(out=f32r(brow_sb), in_=f32r(brow_h[:, :]))
        nc.sync.dma_start(out=f32r(ones_sb), in_=f32r(ones_h[:, :]))

        # persistent through phases 2-3 (80.25 KB/partition)
        qrot_sb = big.tile([128, 8, N], F32)      # Q_rot^T  (d-major)
        krot_sb = big.tile([128, 8, N], F32)      # K_rot^T
        vext_sb = big.tile([128, 8, 16, 65], F32)  # V | ones, per tok-block

        # ============ phases 1-2 (xT scoped here) ============
        with tc.tile_pool(name="xtp", bufs=1) as xtp:
            xT_sb = xtp.tile([128, 8, N], F32)

            # ---- phase 1: load x^T (host-transposed) on the SWDGE
            # queue so it streams in parallel with w_qkv on HWDGE ----
            for cb in range(8):
                nc.gpsimd.dma_start(
                    out=f32r(xT_sb[:, cb, :]),
                    in_=f32r(xt_h[cb * 128:(cb + 1) * 128, :]),
                )

            # ---- phase 2: QKV + rotary + V_ext ----
            with (
                tc.tile_pool(name="wq", bufs=16) as wq,
                tc.tile_pool(name="rot", bufs=3) as rot,
                tc.tile_pool(name="ps_qkv", bufs=3, space="PSUM") as ps_qkv,
                tc.tile_pool(name="ps_misc", bufs=1, space="PSUM") as ps_misc,
            ):
                for og in (4, 5, 0, 2, 1, 3):     # V first, then Q/K interleaved
                    w_tiles = []
                    for kb in range(8):
                        w_t = wq.tile([128, 512], F32, tag="w_t",
                                      name=f"w_t{og}_{kb}")
                        nc.sync.dma_start(
                            out=f32r(w_t),
                            in_=f32r(wqkv_h[kb * 128:(kb + 1) * 128,
                                            og * 512:(og + 1) * 512]),
                        )
                        w_tiles.append(w_t)

                    if og < 4:                    # Q^T / K^T (d-major)
                        for j in range(4):
                            ob = og * 4 + j       # global 128-out block
                            qkv_ps = ps_qkv.tile([128, N], F32, tag="qkv_ps", name=f"qkv_ps{ob}")
                            for kb in range(8):
                                lhs = w_tiles[kb][:, j * 128:(j + 1) * 128]
                                for qc in range(2):
                                    nc.tensor.matmul(
                                        qkv_ps[:, qc * 512:(qc + 1) * 512],
                                        f32r(lhs),
                                        f32r(xT_sb[:, kb,
                                                   qc * 512:(qc + 1) * 512]),
                                        start=(kb == 0),
                                        stop=(kb == 7),
                                    )
                            dst = (qrot_sb if ob < 8 else krot_sb)
                            hp = ob % 8
                            q_sb = rot.tile([128, N], F32, tag="q_sb")
                            nc.scalar.copy(f32r(q_sb), qkv_ps)
                            shuf_ps = ps_misc.tile([128, N], F32, tag="shuf_ps",
                                                   name=f"shuf{ob}")
                            for qc in range(2):
                                nc.tensor.matmul(
                                    shuf_ps[:, qc * 512:(qc + 1) * 512],
                                    f32r(pshuf_sb),
                                    f32r(q_sb[:, qc * 512:(qc + 1) * 512]),
                                    start=True,
                                    stop=True,
                                )
                            tmp = rot.tile([128, N], F32, tag="tmp")
                            nc.vector.tensor_mul(tmp, shuf_ps, sin_sb)
                            nc.vector.tensor_mul(f32r(dst[:, hp, :]), q_sb, cos_sb)
                            nc.vector.tensor_add(
                                f32r(dst[:, hp, :]), dst[:, hp, :], tmp
                            )
                    else:                         # V half (token-major)
                        vh = og - 4               # 0: heads 0-7, 1: 8-15
                        for tb in range(8):
                            v_ps = ps_qkv.tile([128, 512], F32, tag="qkv_ps", name=f"v_ps{og}_{tb}")
                            for kb in range(8):
                                nc.tensor.matmul(
                                    v_ps,
                                    f32r(xT_sb[:, kb,
                                               tb * 128:(tb + 1) * 128]),
                                    f32r(w_tiles[kb]),
                                    start=(kb == 0),
                                    stop=(kb == 7),
                                )
                            nc.scalar.copy(
                                f32r(vext_sb[:, tb, vh * 8:(vh + 1) * 8, 0:64]),
                                v_ps.rearrange("p (a b) -> p a b", a=8),
                            )
                        if vh == 1:
                            for tb in range(8):
                                nc.sync.dma_start(
                                    out=f32r(vext_sb[:, tb, :, 64:65]),
                                    in_=f32r(onescol_h[:, :]),
                                )

        # ============ phases 3-4 (attn scoped here) ============
        with tc.tile_pool(name="attnp", bufs=1) as attnp:
            attn_sb = attnp.tile([128, 8, N], F32)   # attn_out^T (c-major)

            # ---- phase 3: attention, head pairs (adjacent K=64 matmuls
            # at base partitions 0/64 row-pack on the PE) ----
            with tc.tile_pool(name="wpre", bufs=8) as wpre:
                # prefetch w_proj rows during attention
                wp_tiles = []
                for cb in range(8):
                    wp_t = wpre.tile([128, C], F32, tag="wp_t", name=f"wp{cb}")
                    nc.sync.dma_start(
                        out=f32r(wp_t),
                        in_=f32r(wproj_h[cb * 128:(cb + 1) * 128, :]),
                    )
                    wp_tiles.append(wp_t)

                with (
                    tc.tile_pool(name="expp", bufs=4) as expp,
                    tc.tile_pool(name="navp", bufs=2) as navp,
                    tc.tile_pool(name="ps_lg", bufs=2, space="PSUM") as ps_lg,
                    tc.tile_pool(name="ps_av", bufs=2, space="PSUM") as ps_av,
                ):
                    def emit_logits(h, kt):
                        hp, r0 = h // 2, (h % 2) * 64
                        lg_ps = ps_lg.tile([128, N], F32, tag="lg_ps",
                                           name=f"lg{h}_{kt}")
                        lhs = krot_sb[r0:r0 + 64, hp,
                                      kt * 128:(kt + 1) * 128]
                        for qc in range(2):
                            nc.tensor.matmul(
                                lg_ps[:, qc * 512:(qc + 1) * 512],
                                f32r(lhs),
                                f32r(qrot_sb[r0:r0 + 64, hp,
                                             qc * 512:(qc + 1) * 512]),
                                start=True,
                                stop=True,
                            )
                        return lg_ps

                    lg_next = None
                    for h in range(H):
                        hp, r0 = h // 2, (h % 2) * 64
                        av_ps = ps_av.tile([65, N], F32, tag="av_ps",
                                           name=f"av{h}")
                        for kt in range(8):
                            if lg_next is not None:
                                lg_ps, lg_next = lg_next, None
                            else:
                                lg_ps = emit_logits(h, kt)
                            if kt == 7 and h + 1 < H:
                                # pre-issue next head's first logits so the
                                # ACT pipe never drains at head boundaries
                                lg_next = emit_logits(h + 1, 0)
                            e_sb = expp.tile([128, N], F32, tag="e_sb",
                                             name=f"e{h}_{kt}")
                            nc.scalar.activation(
                                f32r(e_sb), lg_ps,
                                mybir.ActivationFunctionType.Exp, scale=0.125,
                            )
                            for qc in range(2):
                                nc.tensor.matmul(
                                    av_ps[:, qc * 512:(qc + 1) * 512],
                                    f32r(vext_sb[:, kt, h, :]),
                                    f32r(e_sb[:, qc * 512:(qc + 1) * 512]),
                                    start=(kt == 0),
                                    stop=(kt == 7),
                                )
                        recip = navp.tile([1, N], F32, tag="recip", bufs=1)
                        nc.vector.reciprocal(recip, av_ps[64:65, :])
                        av_sb = navp.tile([64, N], F32, tag="av_sb")
                        nc.vector.tensor_copy(av_sb, av_ps[0:64, :])
                        # broadcast 1/rowsum across partitions on gpsimd
                        rb_sb = navp.tile([64, N], F32, tag="rb_sb", bufs=1)
                        nc.gpsimd.partition_broadcast(rb_sb, recip)
                        nc.vector.tensor_mul(
                            f32r(attn_sb[r0:r0 + 64, hp, :]), av_sb, rb_sb
                        )

                # ---- phase 4: proj + bias (weights prefetched) ----
                with (
                    tc.tile_pool(name="yout", bufs=2) as yout,
                    tc.tile_pool(name="ps_y", bufs=4, space="PSUM") as ps_y,
                ):
                    for qg in range(2):               # 4 q-blocks per group
                        y_tiles = [
                            ps_y.tile([128, C], F32, tag="y_ps",
                                      name=f"y_ps{qg}_{i}")
                            for i in range(4)
                        ]
                        for cb in range(8):
                            for i in range(4):
                                qb = qg * 4 + i
                                lhs = attn_sb[:, cb, qb * 128:(qb + 1) * 128]
                                for oc in range(2):
                                    nc.tensor.matmul(
                                        y_tiles[i][:, oc * 512:(oc + 1) * 512],
                                        f32r(lhs),
                                        f32r(wp_tiles[cb][:,
                                             oc * 512:(oc + 1) * 512]),
                                        start=(cb == 0),
                                        stop=False,
                                    )
                        for i in range(4):
                            qb = qg * 4 + i
                            for oc in range(2):
                                nc.tensor.matmul(
                                    y_tiles[i][:, oc * 512:(oc + 1) * 512],
                                    f32r(ones_sb),
                                    f32r(brow_sb[:, oc * 512:(oc + 1) * 512]),
                                    start=False,
                                    stop=True,
                                )
                            y_sb = yout.tile([128, C], F32, tag="y_sb")
                            nc.scalar.copy(y_sb, y_tiles[i])
                            nc.sync.dma_start(
                                out=out_h[qb * 128:(qb + 1) * 128, :], in_=y_sb
                            )
    nc.finalize()
    return nc


_PROGRAM = None


def kernel(x, w_qkv, w_proj, b_proj):
    global _PROGRAM
    if _PROGRAM is None:
        _PROGRAM = _build_program()
    nc = _PROGRAM

    cos_d, sin_d, pshuf = _host_tables()
    shared = {
        "w_qkv": np.ascontiguousarray(w_qkv, np.float32),
        "w_proj": np.ascontiguousarray(w_proj, np.float32),
        "b_row": np.ascontiguousarray(b_proj, np.float32).reshape(1, C),
        "cos_d": cos_d,
        "sin_d": sin_d,
        "pshuf": pshuf,
        "ones_row": np.ones((1, 128), np.float32),
        "ones_col": np.ones((128, 16), np.float32),
    }
    in_maps = [
        {"xt": np.ascontiguousarray(np.asarray(x[b], np.float32).T), **shared}
        for b in range(NCORES)
    ]
    res = run_bass_kernel_spmd(nc, in_maps, core_ids=list(range(NCORES)))
    return np.stack([res.results[b]["out"] for b in range(NCORES)], axis=0)


if __name__ == "__main__":
    xs = np.random.randn(B, N, C).astype(np.float32)
    wq = (np.random.randn(C, 3 * C) / np.sqrt(C)).astype(np.float32)
    wp = (np.random.randn(C, C) / np.sqrt(C)).astype(np.float32)
    bp = (np.random.randn(C) * 0.01).astype(np.float32)
    out = kernel(x=xs, w_qkv=wq, w_proj=wp, b_proj=bp)
    print(out.shape, out.dtype)

